# revision 40
# baseline (speedup 1.0000x reference)
"""CRF-RNN mean-field kernel for Trainium2 (8 NeuronCores, data-parallel over T).

Math: reference computes, with x0 = inputs @ W_feat.T (T,N),
A[i,j] = sum_k kernels[i,j,k] W_lin[k], denom[i] = sum(W_feat) + 2*sum_j A[i,j],
the 4-step recurrence  x <- (x0 + 2 x A^T) / denom.
The recurrence is linear, so with D = diag(1/denom), B = 2 A^T D:
    x4 = x0 @ E,   E = D (I + B + B^2 + B^3) + B^4     (256x256, precomputed on-chip)
Layout: T sharded 8 ways; kernels sharded by i-rows (0.5MB/core) with an
AllGather of the (256,256) A matrix; E built on every core via PE transposes
and 12 small fp32 matmuls. Main stream per core: 8x 2MB input blocks; a custom
fused DVE op (running weighted sum) + strided differences produce x0 in one
pass per block; PE transposes x0 and accumulates two fp32 matmuls against E;
stores go out on the scalar HWDGE ring while loads use sync/gpsimd rings.
"""

import os
import sys

for _p in ("/opt/trn_rl_repo",):
    if _p not in sys.path and os.path.isdir(_p):
        sys.path.insert(0, _p)

import numpy as np

import concourse.bass as bass
import concourse.mybir as mybir
from concourse import bacc
from concourse.bass_utils import run_bass_kernel_spmd
from concourse.masks import make_identity
from concourse.tile import TileContext

F32 = mybir.dt.float32
AL = mybir.AluOpType
AX = mybir.AxisListType


def _register_scanmul():
    """Custom DVE op: out = running_sum(Src0 * Src1) along the free dim.
    Used to fuse the weight-multiply and the m/k-contraction into one pass;
    group sums are then strided differences of the running sum."""
    import concourse.dve_ops as dve_ops
    from concourse.dve_ops import DveOp
    from concourse.dve_spec import AluOp, Spec, Src0, Src1, lower, scan
    from concourse.dve_uop import DveOpSpec

    if hasattr(dve_ops, "TENSOR_SCANMUL_ANT"):
        return dve_ops.TENSOR_SCANMUL_ANT

    def ref(in0, in1, s0, s1, imm2):
        a = np.asarray(in0, np.float32)
        b = np.asarray(in1, np.float32).reshape(a.shape)
        return np.cumsum(a * b, axis=-1, dtype=np.float32)

    name = "TENSOR_SCANMUL_ANT"
    spec = Spec(body=scan(AluOp.ADD, Src0 * Src1), reference=ref)
    row = max(dve_ops._SUB_OPCODE_FOR_NAME.values()) + 1
    assert row < 0x20, "custom-DVE opcode rows exhausted"
    # Pin the sha self-consistently (HW-validated for these uops); computed
    # here so the registration survives upstream OPS-list changes.
    shas = {}
    for ver in ("v3", "v4"):
        try:
            shas[ver] = DveOpSpec(
                name=name, opcode=row, uops=lower(spec, ver=ver), rd1_en=True
            ).sha(ver)
        except Exception:
            pass
    op = DveOp(name, spec, subdim=False, uops_sha=shas)
    dve_ops.OPS.append(op)
    dve_ops._SUB_OPCODE_FOR_NAME[op.name] = row
    dve_ops.CUSTOM_DVE_SPECS[op.name] = op.spec
    dve_ops.TENSOR_SCANMUL_ANT = op
    return op

T, N, M, K = 16384, 256, 8, 16
NCORES = 8
TL = T // NCORES  # 2048 rows per core
P = 128
NT = TL // P  # 16 t-tiles per core
NH = N // P  # 2 region halves
BL = 4 * P  # t-rows per DMA block (4 psum-sized subtiles, 4MB loads)
NB = TL // BL  # 4 blocks per core
SCN = (BL // P) * N * M  # 8192 scanned elems per partition per block


def _kernel_body(tc, inp, kern, wf, wl, out, mode="full", stream_loop_cm=None):
    """mode: 'full' | 'dma' (loads+stores only) | 'x0' (dma + MAC chains)
    | 'pe' (dma + transpose/matmul path, x0 faked from raw input).
    stream_loop_cm: optional contextmanager factory wrapping the main stream
    (used by the timing harness to For_i-loop it; collectives can't loop)."""
    nc = tc.nc
    scanmul = _register_scanmul()

    import contextlib
    import dataclasses
    from contextlib import ExitStack

    with ExitStack() as ctx:
        const = ctx.enter_context(tc.tile_pool(name="const", bufs=1))
        work = ctx.enter_context(tc.tile_pool(name="work", bufs=2))
        srp = ctx.enter_context(tc.tile_pool(name="srp", bufs=1))
        x0p = ctx.enter_context(tc.tile_pool(name="x0p", bufs=3))
        outp = ctx.enter_context(tc.tile_pool(name="outp", bufs=3))
        pst = ctx.enter_context(tc.tile_pool(name="pst", bufs=2, space="PSUM"))
        pso = ctx.enter_context(tc.tile_pool(name="pso", bufs=2, space="PSUM"))

        # ---------------- constants ----------------
        ident = const.tile([P, P], F32)
        make_identity(nc, ident[:])

        wf_row = const.tile([1, M], F32)
        nc.sync.dma_start(wf_row[:], wf[:, :])
        wf_sb = const.tile([P, M], F32)
        nc.gpsimd.partition_broadcast(wf_sb[:], wf_row[:])

        wl_row = const.tile([1, K], F32)
        nc.sync.dma_start(wl_row[:], wl[:, :])
        wl_sb = const.tile([P, K], F32)
        nc.gpsimd.partition_broadcast(wl_sb[:], wl_row[:])

        fw_sum = const.tile([P, 1], F32)
        nc.vector.tensor_reduce(fw_sum[:], wf_sb[:], axis=AX.X, op=AL.add)

        # stride-0 repeat views of the weight rows (no SBUF cost)
        wrep = dataclasses.replace(
            wf_sb[:], ap=[wf_sb[:].ap[0], [0, SCN // M], [1, M]]
        )  # (128, 512, 8): wf repeated along the block scan stream
        wlrep = dataclasses.replace(
            wl_sb[:], ap=[wl_sb[:].ap[0], [0, N], [1, K]]
        )  # (128, 256, 16): wl repeated along the kern scan stream

        # ------------- precompute E (every core, identical) -------------
        E = [const.tile([P, N], F32, tag=f"E{jb}", name=f"E{jb}") for jb in range(NH)]
        NSH = N // NCORES  # 32 kern rows handled by this core
        if mode != "full":
            # same (sharded) kern DMA traffic, fake E, skip the A/E computation
            kt = const.tile([NSH, N * K], F32, tag="kernsl", name="kern_sb")
            nc.gpsimd.dma_start(kt[:], kern[:, :])
            if mode == "pe":
                for jb in range(NH):
                    nc.gpsimd.memset(E[jb][:], 0.001)
        else:
            # A[i,j] = sum_k kern[i,j,k] * wl[k], computed as strided differences
            # of a running weighted sum. Each core computes only its 32 i-rows
            # from its kern slice; an AllGather distributes the full (256,256) A.
            kt = const.tile([NSH, N * K], F32, tag="kernsl", name="kern_sb")
            nc.gpsimd.dma_start(kt[:], kern[:, :])
            krun = srp.tile([P, SCN + 16], F32, tag="srun", name="krun")
            nc.gpsimd.memset(krun[:NSH, 0:1], 0.0)
            nc.vector._custom_dve(
                scanmul,
                out=krun[:NSH, 1 : N * K + 1],
                in0=kt[:],
                in1=dataclasses.replace(
                    wl_sb[:NSH, :], ap=[wl_sb[:NSH, :].ap[0], [0, N], [1, K]]
                ),
            )
            vA = krun[:NSH, K : N * K + K].rearrange("p (j k) -> p j k", k=K)[:, :, 0]
            vB = krun[:NSH, 0 : N * K].rearrange("p (j k) -> p j k", k=K)[:, :, 0]
            A_small = const.tile([NSH, N], F32, tag="A_small", name="A_small")
            nc.vector.tensor_sub(A_small[:], vA, vB)

            dram = ctx.enter_context(tc.tile_pool(name="dram", bufs=1, space="DRAM"))
            ag_in = dram.tile([NSH, N], F32, name="ag_in")
            ag_out = dram.tile([N, N], F32, name="ag_out")
            nc.gpsimd.dma_start(ag_in[:], A_small[:])
            nc.gpsimd.collective_compute(
                "AllGather",
                AL.bypass,
                replica_groups=[list(range(NCORES))],
                ins=[ag_in.opt()],
                outs=[ag_out.opt()],
            )

            Bt = []  # Bt[h][i_loc, j] = B[j, h*128+i_loc] = 2*invd[i]*A[i,j]
            invd = []  # [128,1] per half, partition index = region index
            for h in range(NH):
                Ah = const.tile([P, N], F32, tag=f"A{h}", name=f"A{h}")
                nc.sync.dma_start(Ah[:], ag_out[h * P : (h + 1) * P, :])
                red = const.tile([P, 1], F32, tag=f"red{h}", name=f"red{h}")
                nc.vector.tensor_reduce(red[:], Ah[:], axis=AX.X, op=AL.add)
                den = const.tile([P, 1], F32, tag=f"den{h}", name=f"den{h}")
                nc.vector.scalar_tensor_tensor(
                    den[:], red[:], 2.0, fw_sum[:], op0=AL.mult, op1=AL.add
                )
                inv = const.tile([P, 1], F32, tag=f"invd{h}", name=f"invd{h}")
                nc.vector.reciprocal(inv[:], den[:])
                invd.append(inv)
                inv2 = const.tile([P, 1], F32, tag=f"invd2{h}", name=f"invd2{h}")
                nc.vector.tensor_scalar_mul(inv2[:], inv[:], 2.0)
                Bth = const.tile([P, N], F32, tag=f"Bt{h}", name=f"Bt{h}")
                nc.scalar.mul(Bth[:], Ah[:], inv2[:, 0:1])
                Bt.append(Bth)

            # B1[jb][j_loc, i] = B[jb*128+j_loc, i]  (PE transpose of Bt blocks)
            B1 = [
                const.tile([P, N], F32, tag=f"B1{jb}", name=f"B1{jb}")
                for jb in range(NH)
            ]
            for jb in range(NH):
                for ih in range(NH):
                    pt = pst.tile([P, P], F32, tag="tr", name=f"trB{jb}{ih}")
                    nc.tensor.transpose(
                        pt[:], Bt[ih][:, jb * P : (jb + 1) * P], ident[:]
                    )
                    nc.scalar.copy(B1[jb][:, ih * P : (ih + 1) * P], pt[:])

            # Powers: B_{n+1}[j,i] = sum_l Bt[l,j] * B_n[l,i]
            def mat_next(rhs_tiles, tag):
                res = [
                    const.tile([P, N], F32, tag=f"{tag}{jb}", name=f"{tag}{jb}")
                    for jb in range(NH)
                ]
                for jb in range(NH):
                    ps = pso.tile([P, N], F32, tag="pw", name=f"pw{tag}{jb}")
                    for lh in range(NH):
                        nc.tensor.matmul(
                            ps[:],
                            Bt[lh][:, jb * P : (jb + 1) * P],
                            rhs_tiles[lh][:],
                            start=(lh == 0),
                            stop=(lh == NH - 1),
                        )
                    nc.scalar.copy(res[jb][:], ps[:])
                return res

            B2 = mat_next(B1, "B2")
            B3 = mat_next(B2, "B3")
            B4 = mat_next(B3, "B4")

            # E[jb] = invd ⊙ (I + B1 + B2 + B3)[jb] + B4[jb]
            for jb in range(NH):
                s = E[jb]
                nc.vector.tensor_add(s[:], B1[jb][:], B2[jb][:])
                nc.vector.tensor_add(s[:], s[:], B3[jb][:])
                nc.vector.tensor_add(
                    s[:, jb * P : (jb + 1) * P], s[:, jb * P : (jb + 1) * P], ident[:]
                )
                nc.scalar.mul(s[:], s[:], invd[jb][:, 0:1])
                nc.vector.tensor_add(s[:], s[:], B4[jb][:])

        # ------------- main loop: blocks of 256 t-rows -------------
        NQ = BL // P  # 2 psum-subtiles per block
        stream_cm = stream_loop_cm() if stream_loop_cm else contextlib.nullcontext()
        ctx.enter_context(stream_cm)
        for b in range(NB):
            it2 = work.tile([P, SCN], F32, tag="in", name=f"in{b}")
            src = inp[b * BL : (b + 1) * BL, :].rearrange("(q p) f -> p q f", p=P)
            # alternate the input loads between the two DGE issue paths
            ld_eng = nc.sync if b % 2 == 0 else nc.gpsimd
            ld_eng.dma_start(it2[:].rearrange("p (q f) -> p q f", q=NQ), src)

            if mode in ("full", "x0"):
                srun = srp.tile([P, SCN + 16], F32, tag="srun", name=f"srun{b}")
                nc.gpsimd.memset(srun[:, 0:1], 0.0)
                nc.vector._custom_dve(
                    scanmul, out=srun[:, 1 : SCN + 1], in0=it2[:], in1=wrep
                )
                vA = srun[:, M : SCN + M].rearrange("p (g m) -> p g m", m=M)[:, :, 0]
                vB = srun[:, 0:SCN].rearrange("p (g m) -> p g m", m=M)[:, :, 0]
                x0b = x0p.tile([P, NQ * N], F32, tag="x0", name=f"x0{b}")
                nc.vector.tensor_sub(x0b[:], vA, vB)
            elif mode == "pe":
                x0b = it2[:, 0 : NQ * N]
            else:
                x0b = None

            if mode in ("full", "pe"):
                ot2 = outp.tile([P, NQ * N], F32, tag="ot", name=f"ot{b}")
                for q in range(NQ):
                    x0T = []
                    for jb in range(NH):
                        pt = pst.tile([P, P], F32, tag="tr", name=f"tr{b}_{q}{jb}")
                        nc.tensor.transpose(
                            pt[:],
                            x0b[:, q * N + jb * P : q * N + (jb + 1) * P],
                            ident[:],
                        )
                        xs = x0p.tile(
                            [P, P], F32, tag=f"x0T{jb}", name=f"x0T{b}_{q}{jb}"
                        )
                        nc.scalar.copy(xs[:], pt[:])
                        x0T.append(xs)
                    po = pso.tile([P, N], F32, tag="out", name=f"po{b}_{q}")
                    for jb in range(NH):
                        nc.tensor.matmul(
                            po[:],
                            x0T[jb][:],
                            E[jb][:],
                            start=(jb == 0),
                            stop=(jb == NH - 1),
                        )
                    nc.scalar.copy(ot2[:, q * N : (q + 1) * N], po[:])
            elif mode == "x0":
                ot2 = x0b
            else:  # dma
                ot2 = outp.tile([P, NQ * N], F32, tag="ot", name=f"ot{b}")
                nc.gpsimd.memset(ot2[:], 0.0)
            dst = out[b * BL : (b + 1) * BL, :].rearrange("(q p) i -> p q i", p=P)
            nc.scalar.dma_start(dst, ot2[:].rearrange("p (q i) -> p q i", q=NQ))


_NC_CACHE = {}


def _build(bodies=1):
    if bodies in _NC_CACHE:
        return _NC_CACHE[bodies]
    nc = bacc.Bacc(
        "TRN2",
        target_bir_lowering=False,
        debug=False,
        enable_asserts=False,
        num_devices=NCORES,
    )
    inp = nc.dram_tensor("inp", (TL, N * M), F32, kind="ExternalInput").ap()
    kern = nc.dram_tensor("kern", (N // NCORES, N * K), F32, kind="ExternalInput").ap()
    wf = nc.dram_tensor("wf", (1, M), F32, kind="ExternalInput").ap()
    wl = nc.dram_tensor("wl", (1, K), F32, kind="ExternalInput").ap()
    out = nc.dram_tensor("out", (TL, N), F32, kind="ExternalOutput").ap()
    with TileContext(nc) as tc:
        for _ in range(bodies):
            _kernel_body(tc, inp, kern, wf, wl, out)
    nc.compile()
    _NC_CACHE[bodies] = nc
    return nc


def _build_loop(mode="full"):
    """Variant with the body inside a dynamic For_i whose bound comes from the
    int32 input `reps` — one executable, runtime-varied body count, for timing."""
    key = ("loop", mode)
    if key in _NC_CACHE:
        return _NC_CACHE[key]
    nc = bacc.Bacc(
        "TRN2",
        target_bir_lowering=False,
        debug=False,
        enable_asserts=False,
        num_devices=NCORES,
    )
    inp = nc.dram_tensor("inp", (TL, N * M), F32, kind="ExternalInput").ap()
    kern = nc.dram_tensor("kern", (N // NCORES, N * K), F32, kind="ExternalInput").ap()
    wf = nc.dram_tensor("wf", (1, M), F32, kind="ExternalInput").ap()
    wl = nc.dram_tensor("wl", (1, K), F32, kind="ExternalInput").ap()
    reps = nc.dram_tensor("reps", (1, 1), mybir.dt.int32, kind="ExternalInput").ap()
    out = nc.dram_tensor("out", (TL, N), F32, kind="ExternalOutput").ap()
    with TileContext(nc) as tc:
        with tc.tile_pool(name="repsp", bufs=1) as rp:
            reps_sb = rp.tile([1, 1], mybir.dt.int32)
            nc.sync.dma_start(reps_sb[:], reps[:, :])
            r_val = nc.values_load(
                reps_sb[:], min_val=0, max_val=256, skip_runtime_bounds_check=True
            )
            # Precompute (incl. the AllGather, which cannot sit inside a
            # dynamic loop) runs once; only the main stream is looped R times.
            _kernel_body(
                tc, inp, kern, wf, wl, out, mode=mode,
                stream_loop_cm=lambda: tc.For_i(0, r_val, 1),
            )
    nc.compile()
    _NC_CACHE[key] = nc
    return nc


def bench_loop(rvals=(1, 501), reps=16, mode="full"):
    """Time one executable at different runtime body counts R; per-dispatch
    offsets cancel in the R-slope. Synchronizes by fetching output values
    (block_until_ready under axon does not wait for device completion)."""
    import time

    import jax

    rng = np.random.default_rng(0)
    inp = rng.standard_normal((T, N * M), dtype=np.float32)
    kr = rng.random((N, N * K), dtype=np.float32)
    wf = (rng.random((1, M), dtype=np.float32) * 0.01).astype(np.float32)
    wl = (rng.random((1, K), dtype=np.float32) * 0.01).astype(np.float32)

    nc = _build_loop(mode)
    fn, in_names, out_names, out_avals, sh = _pjrt_callable(nc)
    times = {}
    for rv in rvals:
        cat = {
            "inp": inp,
            "kern": kr,
            "wf": np.tile(wf, (NCORES, 1)),
            "wl": np.tile(wl, (NCORES, 1)),
            "reps": np.full((NCORES, 1), rv, np.int32),
        }
        args = [jax.device_put(cat[n], sh) for n in in_names]
        args += [
            jax.device_put(np.zeros((NCORES * a.shape[0], *a.shape[1:]), a.dtype), sh)
            for a in out_avals
        ]
        o = fn(*args)
        np.asarray(o[0])
        ts = []
        for _ in range(reps):
            t0 = time.perf_counter()
            o = fn(*args)
            np.asarray(o[0])  # forced sync via value fetch
            ts.append(time.perf_counter() - t0)
        ts.sort()
        med = ts[len(ts) // 2]
        times[rv] = med
        print(f"R={rv}: median {med*1e3:.2f} ms (min {ts[0]*1e3:.2f})")
    rs = sorted(times)
    slope_ns = None
    if len(rs) >= 2:
        slope_ns = (times[rs[-1]] - times[rs[0]]) / (rs[-1] - rs[0]) * 1e9
        print(f"per-body time (R={rs[-1]} vs R={rs[0]}): {slope_ns:.0f} ns")
    return slope_ns, times


def kernel(inputs, kernels, W_feat, W_lin, trace=False):
    inp = np.ascontiguousarray(np.asarray(inputs, dtype=np.float32).reshape(T, N * M))
    kr = np.ascontiguousarray(np.asarray(kernels, dtype=np.float32).reshape(N, N * K))
    wf = np.ascontiguousarray(np.asarray(W_feat, dtype=np.float32).reshape(1, M))
    wl = np.ascontiguousarray(np.asarray(W_lin, dtype=np.float32).reshape(1, K))

    nc = _build(1)
    in_maps = [
        {
            "inp": inp[c * TL : (c + 1) * TL],
            "kern": kr[c * (N // NCORES) : (c + 1) * (N // NCORES)],
            "wf": wf,
            "wl": wl,
        }
        for c in range(NCORES)
    ]
    res = run_bass_kernel_spmd(nc, in_maps, core_ids=list(range(NCORES)), trace=trace)
    outs = [res.results[c]["out"] for c in range(NCORES)]
    full = np.concatenate(outs, axis=0).reshape(T, N, 1)
    if trace:
        kernel.last_exec_time_ns = res.exec_time_ns
        kernel.last_results = res
    return full


def _pjrt_callable(nc):
    """Build a jit(shard_map(bass_exec)) callable + device-resident input list,
    mirroring bass2jax.run_bass_via_pjrt (no donation: outputs reallocated)."""
    import jax
    from jax.sharding import Mesh, NamedSharding, PartitionSpec
    from jax.experimental.shard_map import shard_map

    from concourse.bass2jax import (
        _bass_exec_p,
        install_neuronx_cc_hook,
        partition_id_tensor,
    )

    install_neuronx_cc_hook()
    partition_name = nc.partition_id_tensor.name if nc.partition_id_tensor else None
    in_names, out_names, out_avals = [], [], []
    for alloc in nc.m.functions[0].allocations:
        if not isinstance(alloc, mybir.MemoryLocationSet):
            continue
        name = alloc.memorylocations[0].name
        if alloc.kind == "ExternalInput":
            if name != partition_name:
                in_names.append(name)
        elif alloc.kind == "ExternalOutput":
            out_names.append(name)
            out_avals.append(
                jax.core.ShapedArray(tuple(alloc.tensor_shape), mybir.dt.np(alloc.dtype))
            )
    all_in = list(in_names) + list(out_names)
    if partition_name is not None:
        all_in.append(partition_name)
    all_in = tuple(all_in)

    def _body(*args):
        operands = list(args)
        if partition_name is not None:
            operands.append(partition_id_tensor())
        return tuple(
            _bass_exec_p.bind(
                *operands,
                out_avals=tuple(out_avals),
                in_names=all_in,
                out_names=tuple(out_names),
                lowering_input_output_aliases=(),
                sim_require_finite=True,
                sim_require_nnan=True,
                nc=nc,
            )
        )

    devices = jax.devices()[:NCORES]
    mesh = Mesh(np.asarray(devices), ("core",))
    nin = len(in_names) + len(out_names)
    fn = jax.jit(
        shard_map(
            _body,
            mesh=mesh,
            in_specs=(PartitionSpec("core"),) * nin,
            out_specs=(PartitionSpec("core"),) * len(out_names),
            check_rep=False,
        )
    )
    sh = NamedSharding(mesh, PartitionSpec("core"))
    return fn, in_names, out_names, out_avals, sh


def bench(bodies_list=(1, 4), reps=30):
    """Time the NEFF via repeated dispatch of R-body program variants.
    Marginal per-body time = (t(R2)-t(R1))/(R2-R1) cancels dispatch overhead."""
    import time

    import jax

    rng = np.random.default_rng(0)
    inp = rng.standard_normal((T, N * M), dtype=np.float32)
    kr = rng.random((N, N * K), dtype=np.float32)
    wf = (rng.random((1, M), dtype=np.float32) * 0.01).astype(np.float32)
    wl = (rng.random((1, K), dtype=np.float32) * 0.01).astype(np.float32)
    vals = {"inp": inp, "kern": kr, "wf": wf, "wl": wl}

    times = {}
    for bodies in bodies_list:
        nc = _build(bodies)
        fn, in_names, out_names, out_avals, sh = _pjrt_callable(nc)
        cat = {
            "inp": inp,
            "kern": kr,
            "wf": np.concatenate([wf] * NCORES, 0),
            "wl": np.concatenate([wl] * NCORES, 0),
        }
        args = [jax.device_put(cat[n], sh) for n in in_names]
        args += [
            jax.device_put(np.zeros((NCORES * a.shape[0], *a.shape[1:]), a.dtype), sh)
            for a in out_avals
        ]
        o = fn(*args)
        jax.block_until_ready(o)  # warm (NEFF compile happens here)
        ts = []
        for _ in range(reps):
            t0 = time.perf_counter()
            o = fn(*args)
            jax.block_until_ready(o)
            ts.append(time.perf_counter() - t0)
        ts.sort()
        med = ts[len(ts) // 2]
        times[bodies] = med
        print(f"bodies={bodies}: median dispatch {med*1e6:.1f} us  (min {ts[0]*1e6:.1f})")
    bs = sorted(times)
    if len(bs) >= 2:
        r1, r2 = bs[0], bs[-1]
        marginal = (times[r2] - times[r1]) / (r2 - r1)
        print(f"marginal per-body time: {marginal*1e9:.0f} ns")
        return marginal * 1e9, times
    return None, times
